# revision 2
# baseline (speedup 1.0000x reference)
"""GCN message-passing kernel for Trainium2, 8 NeuronCores — v3.

Structure vs v2.6:
  - table rows permuted group-major into 4 source groups; per layer the
    AllGather is split into 4 sub-collectives, one per group
  - edges chunked per (dst block, src group) with group-relative int16
    indices (no lo/hi split); gathers for src group g of layer l+1 only
    depend on sub-AG g of layer l -> cross-layer pipelining, no barriers
  - indicators precomputed on host and streamed (no DVE build)
  - L1 input is a host pre-expanded edge stream (no gather at all)
  - decode labels bucketed by (a-group, b-group) 16 ways, consuming z3
    sub-AGs as they land
"""

import numpy as np
import ml_dtypes

BF16 = ml_dtypes.bfloat16

P = 128
N_CORES = 8
N_GROUPS = 4   # source groups == sub-AllGather groups
GG_BLKS = 4    # dst blocks per gather batch
DTC = 32       # decode tile chunks


# ---------------------------------------------------------------- host prep

def make_layout(bpc):
    # last group smallest: its AllGather is the exposed tail each layer
    base = -(-bpc // N_GROUPS)
    last = bpc - (N_GROUPS - 1) * base
    sizes = np.array([base] * (N_GROUPS - 1) + [last])
    gstart = np.concatenate([[0], np.cumsum(sizes)[:-1]])          # local blk
    grp_off = np.concatenate([[0], np.cumsum(N_CORES * sizes * P)[:-1]])
    bounds = np.concatenate([grp_off, [N_CORES * bpc * P]])        # row bounds
    gi = np.zeros(bpc, np.int64)
    for g in range(N_GROUPS):
        gi[gstart[g]:gstart[g] + sizes[g]] = g
    return dict(sizes=sizes, gstart=gstart, grp_off=grp_off, gi=gi,
                bounds=bounds)


def row_of(n, bpc, lay):
    n = np.asarray(n, np.int64)
    blk = n >> 7
    c = blk // bpc
    i = blk % bpc
    g = lay["gi"][i]
    return (lay["grp_off"][g] + c * lay["sizes"][g] * P
            + (i - lay["gstart"][g]) * P + (n & 127))


def group_of_row(r, lay):
    return np.searchsorted(lay["bounds"][1:], np.asarray(r, np.int64),
                           side="right")


def _wrap16(flat_idx):
    t = flat_idx.astype(np.int16).reshape(-1, 16).T
    return np.tile(t, (8, 1))


def chunk_layout(bpc, cnt2d):
    """cnt2d[i][g] chunks per (local block, src group).

    Per gather batch gg (GG_BLKS blocks): for g: for i in gg: chunks.
    """
    n_gg = (bpc + GG_BLKS - 1) // GG_BLKS
    off = np.zeros((bpc, N_GROUPS), np.int64)
    gg_start = np.zeros(n_gg + 1, np.int64)
    call_rng = {}  # (gg, g) -> (chunk0, chunk1)
    pos = 0
    for g2 in range(n_gg):
        gg_start[g2] = pos
        blks = range(g2 * GG_BLKS, min((g2 + 1) * GG_BLKS, bpc))
        for g in range(N_GROUPS):
            p0 = pos
            for i in blks:
                off[i][g] = pos
                pos += int(cnt2d[i][g])
            call_rng[(g2, g)] = (p0, pos)
    gg_start[n_gg] = pos
    return dict(n_gg=n_gg, off=off, gg_start=gg_start, call_rng=call_rng,
                C=int(pos))


def prepare_edges(edge_index, n_nodes, bpc, lay):
    src = np.asarray(edge_index[0], dtype=np.int64)
    dst = np.asarray(edge_index[1], dtype=np.int64)
    deg = np.bincount(dst, minlength=n_nodes).astype(np.float64) + 1.0
    dinv = 1.0 / np.sqrt(deg)

    loops = np.arange(n_nodes, dtype=np.int64)
    esrc = np.concatenate([src, loops])
    edst = np.concatenate([dst, loops])
    enrm = np.concatenate([dinv[src] * dinv[dst], dinv * dinv]).astype(np.float32)

    srow = row_of(esrc, bpc, lay)
    sg = group_of_row(srow, lay)
    blkg = edst >> 7
    dnl = (edst & 127).astype(np.int64)
    n_blocks = N_CORES * bpc

    order = np.lexsort((srow, sg, blkg))
    srow_s = srow[order]
    sg_s = sg[order]
    dnl_s = dnl[order]
    enrm_s = enrm[order]

    cnts = np.bincount(blkg[order] * N_GROUPS + sg_s,
                       minlength=n_blocks * N_GROUPS)
    nbg = cnts.reshape(N_CORES, bpc, N_GROUPS)
    cnt2d = np.ceil(nbg / P).astype(np.int64).max(axis=0)  # [bpc, N_GROUPS]
    zero = cnt2d.sum(axis=1) == 0
    cnt2d[zero, 0] = 1

    cl = chunk_layout(bpc, cnt2d)
    C = cl["C"]

    kuse = np.ones((C,), np.int64)
    for i in range(bpc):
        for g in range(N_GROUPS):
            o0 = int(cl["off"][i][g])
            nmax = nbg[:, i, g]
            for j in range(int(cnt2d[i][g])):
                u = int(np.clip(nmax - j * P, 0, P).max())
                kuse[o0 + j] = max(u, 1)

    gidx = np.zeros((N_CORES, C * P), np.int64)   # group-relative rows
    grow = np.zeros((N_CORES, C * P), np.int64)   # absolute rows
    gdnl = np.zeros((N_CORES, C * P), np.int64)
    gnrm = np.zeros((N_CORES, C * P), np.float32)

    seg_starts = np.zeros(n_blocks * N_GROUPS + 1, np.int64)
    np.cumsum(cnts, out=seg_starts[1:])

    for c in range(N_CORES):
        for i in range(bpc):
            for g in range(N_GROUPS):
                b = (c * bpc + i) * N_GROUPS + g
                s0 = int(seg_starts[b])
                n = int(cnts[b])
                p0 = int(cl["off"][i][g]) * P
                gidx[c, p0:p0 + n] = srow_s[s0:s0 + n] - lay["grp_off"][g]
                grow[c, p0:p0 + n] = srow_s[s0:s0 + n]
                gdnl[c, p0:p0 + n] = dnl_s[s0:s0 + n]
                gnrm[c, p0:p0 + n] = enrm_s[s0:s0 + n]

    eidx = np.stack([_wrap16(gidx[c]) for c in range(N_CORES)])
    inds = np.zeros((N_CORES, P, C * P), BF16)
    for c in range(N_CORES):
        flat = np.zeros((C * P, P), BF16)
        flat[np.arange(C * P), gdnl[c]] = gnrm[c].astype(BF16)
        inds[c] = flat.reshape(C, P, P).transpose(1, 0, 2).reshape(P, C * P)
    return dict(eidx=eidx, inds=inds, cnt2d=cnt2d, C=C, grow=grow, kuse=kuse)


def bucket_order():
    # bucket id k = ((ga*2+pa)*N_GROUPS + gb)*2 + pb; order by max group
    def key(k):
        gb = (k // 2) % N_GROUPS
        ga = k // (2 * N_GROUPS * 2)
        return (max(ga, gb), k)
    return sorted(range(N_GROUPS * 2 * N_GROUPS * 2), key=key)


def prepare_labels(edge_label_index, n_label, bpc, lay):
    a = np.asarray(edge_label_index[0], dtype=np.int64)
    b = np.asarray(edge_label_index[1], dtype=np.int64)
    arow_all = row_of(a, bpc, lay)
    brow_all = row_of(b, bpc, lay)
    ag_all = group_of_row(arow_all, lay)
    bg_all = group_of_row(brow_all, lay)
    per = n_label // N_CORES
    NB = N_GROUPS * 2 * N_GROUPS * 2
    order = bucket_order()
    # relative (to group) row, parity, half-index
    ra = arow_all - lay["grp_off"][ag_all]
    rb = brow_all - lay["grp_off"][bg_all]
    buckets_per_core = []
    for c in range(N_CORES):
        sl = slice(c * per, (c + 1) * per)
        la, lb = ra[sl], rb[sl]
        ag, bg = ag_all[sl], bg_all[sl]
        lab = np.arange(c * per, (c + 1) * per)
        bid = ((ag * 2 + (la & 1)) * N_GROUPS + bg) * 2 + (lb & 1)
        bk = {}
        for k in range(NB):
            sel = bid == k
            o = np.argsort(la[sel], kind="stable")
            bk[k] = (la[sel][o] >> 1, lb[sel][o] >> 1, lab[sel][o])
        buckets_per_core.append(bk)
    tcnt = [max(int(np.ceil(len(buckets_per_core[c][k][0]) / P))
                for c in range(N_CORES)) for k in range(NB)]
    T = sum(tcnt[k] for k in range(NB))
    aidx = np.zeros((N_CORES, T * P), np.int64)
    bidx = np.zeros((N_CORES, T * P), np.int64)
    labmap = np.full((N_CORES, T * P), -1, np.int64)
    for c in range(N_CORES):
        pos = 0
        for k in order:
            la, lb, lab = buckets_per_core[c][k]
            n = len(la)
            cap = tcnt[k] * P
            aidx[c, pos:pos + n] = la
            bidx[c, pos:pos + n] = lb
            labmap[c, pos:pos + n] = lab
            pos += cap
    la_s = np.stack([_wrap16(aidx[c]) for c in range(N_CORES)])
    lb_s = np.stack([_wrap16(bidx[c]) for c in range(N_CORES)])
    return dict(la=la_s, lb=lb_s, tcnt=tcnt, T=T, labmap=labmap)


# ------------------------------------------------------------- device kernel

def build_bass(n_nodes, bpc, cnt2d, tcnt, in_c, hid_c, out_c, lay, kuse):
    from concourse import bacc, bass, mybir
    import concourse.tile as tile

    NPAD = N_CORES * bpc * P
    cl = chunk_layout(bpc, cnt2d)
    C = cl["C"]
    T = int(sum(tcnt))
    f32 = mybir.dt.float32
    bf16 = mybir.dt.bfloat16

    sizes = lay["sizes"]
    gstart = lay["gstart"]
    grp_off = lay["grp_off"]
    gi = lay["gi"]

    nc = bacc.Bacc("TRN2", target_bir_lowering=False, debug=False,
                   num_devices=N_CORES, num_swdge_queues=4)

    xs_d = nc.dram_tensor("xs", [P, C * in_c], bf16, kind="ExternalInput")
    ind_d = nc.dram_tensor("ind", [P, C * P], bf16, kind="ExternalInput")
    w_d = [nc.dram_tensor(f"W{i+1}", s, bf16, kind="ExternalInput")
           for i, s in enumerate([[in_c, hid_c], [hid_c, hid_c], [hid_c, out_c]])]
    b_d = [nc.dram_tensor(f"b{i+1}", [s], bf16, kind="ExternalInput")
           for i, s in enumerate([hid_c, hid_c, out_c])]
    eidx_d = nc.dram_tensor("eidx", [P, C * P // 16], mybir.dt.int16,
                            kind="ExternalInput")
    la_d = nc.dram_tensor("la", [P, T * P // 16], mybir.dt.int16,
                          kind="ExternalInput")
    lb_d = nc.dram_tensor("lb", [P, T * P // 16], mybir.dt.int16,
                          kind="ExternalInput")
    out_d = nc.dram_tensor("out", [P, T], f32, kind="ExternalOutput")

    wdt = [(hid_c, bf16), (hid_c, bf16), (out_c, bf16)]
    zs_d = [[nc.dram_tensor(f"zs{l}_{g}", [int(sizes[g]) * P, w], dt,
                            kind="Internal")
             for g in range(N_GROUPS)] for l, (w, dt) in enumerate(wdt)]
    zf_d = [nc.dram_tensor(f"zf{l}", [NPAD, w], dt, kind="Internal",
                           addr_space="Shared")
            for l, (w, dt) in enumerate(wdt)]

    gq = [0]

    def next_q():
        q = gq[0]
        gq[0] = (q + 1) % 4
        return q

    n_gg = cl["n_gg"]
    gg_start = cl["gg_start"]
    call_rng = cl["call_rng"]
    off = cl["off"]

    with tile.TileContext(nc) as tc:
        with (
            tc.tile_pool(name="consts", bufs=1) as cst,
            tc.tile_pool(name="gath", bufs=3) as gp,
            tc.tile_pool(name="indt", bufs=2) as ip,
            tc.tile_pool(name="dec", bufs=2) as dp,
            tc.tile_pool(name="work", bufs=8) as wp,
            tc.tile_pool(name="outp", bufs=4) as op,
            tc.tile_pool(name="psum", bufs=6, space="PSUM") as ps,
            tc.tile_pool(name="psumz", bufs=2, space="PSUM") as psz,
        ):
            ones1 = cst.tile([1, P], bf16)
            nc.vector.memset(ones1[:], 1.0)

            eidx_sb = cst.tile([P, C * P // 16], mybir.dt.int16)
            nc.sync.dma_start(eidx_sb[:], eidx_d[:, :])
            la_sb = cst.tile([P, T * P // 16], mybir.dt.int16)
            lb_sb = cst.tile([P, T * P // 16], mybir.dt.int16)
            nc.sync.dma_start(la_sb[:], la_d[:, :])
            nc.sync.dma_start(lb_sb[:], lb_d[:, :])

            w_sb = []
            bias_sb = []
            for l in range(3):
                wt = cst.tile([hid_c if l else in_c, out_c if l == 2 else hid_c],
                              bf16)
                nc.sync.dma_start(wt[:], w_d[l][:, :])
                w_sb.append(wt)
                bt = cst.tile([1, out_c if l == 2 else hid_c], bf16)
                nc.sync.dma_start(bt[:], b_d[l][None, :])
                bias_sb.append(bt)

            gg_max = int(max(gg_start[g + 1] - gg_start[g]
                             for g in range(n_gg)))

            for l in range(3):
                oc = out_c if l == 2 else hid_c
                for g2 in range(n_gg):
                    c0 = int(gg_start[g2])
                    c1 = int(gg_start[g2 + 1])
                    cnt = c1 - c0
                    blks = list(range(g2 * GG_BLKS,
                                      min((g2 + 1) * GG_BLKS, bpc)))
                    gt = gp.tile([P, gg_max * in_c], bf16, tag="gath")
                    g3 = gt[:].rearrange("p (c f) -> p c f", c=gg_max)
                    it = ip.tile([P, gg_max * P], bf16, tag="indt")
                    i3 = it[:].rearrange("p (c f) -> p c f", c=gg_max)
                    nc.sync.dma_start(it[:, 0:cnt * P],
                                      ind_d[:, c0 * P:c1 * P])
                    if l == 0:
                        nc.sync.dma_start(
                            gt[:, 0:cnt * in_c],
                            xs_d[:, c0 * in_c:c1 * in_c])
                    else:
                        for g in range(N_GROUPS):
                            r0, r1 = call_rng[(g2, g)]
                            if r1 == r0:
                                continue
                            gr0 = int(grp_off[g])
                            gr1 = gr0 + N_CORES * int(sizes[g]) * P
                            nc.gpsimd.dma_gather(
                                out_ap=g3[:, r0 - c0:r1 - c0, :],
                                in_ap=zf_d[l - 1][gr0:gr1, :],
                                idxs_ap=eidx_sb[:, r0 * 8:r1 * 8],
                                num_idxs=(r1 - r0) * P,
                                num_idxs_reg=(r1 - r0) * P,
                                elem_size=in_c,
                                single_packet=False, queue_num=next_q())

                    for i in blks:
                        kpos = []
                        for g in range(N_GROUPS):
                            kpos += list(range(int(off[i][g]) - c0,
                                               int(off[i][g] + cnt2d[i][g])
                                               - c0))
                        nchunks = len(kpos)
                        agg_ps = ps.tile([P, P], f32, tag="agg", space="PSUM")
                        for j, kp in enumerate(kpos):
                            ku = int(kuse[c0 + kp])
                            nc.tensor.matmul(
                                out=agg_ps[:], lhsT=g3[0:ku, kp, :],
                                rhs=i3[0:ku, kp, :],
                                start=(j == 0), stop=(j == nchunks - 1))

                        aggT = wp.tile([P, P], bf16, tag="aggT")
                        nc.vector.tensor_copy(out=aggT[:], in_=agg_ps[:])

                        z_ps = psz.tile([P, oc], f32, tag="z", space="PSUM")
                        nc.tensor.matmul(out=z_ps[:], lhsT=ones1[:],
                                         rhs=bias_sb[l][:], start=True,
                                         stop=False)
                        nc.tensor.matmul(out=z_ps[:], lhsT=aggT[:],
                                         rhs=w_sb[l][:], start=False,
                                         stop=True)

                        g = int(gi[i])
                        ig = i - int(gstart[g])
                        z_sb = op.tile([P, oc], bf16, tag="z_sb")
                        if l < 2:
                            nc.scalar.activation(
                                out=z_sb[:], in_=z_ps[:],
                                func=mybir.ActivationFunctionType.Relu)
                        else:
                            nc.vector.tensor_copy(out=z_sb[:], in_=z_ps[:])
                        nc.sync.dma_start(
                            zs_d[l][g][ig * P:(ig + 1) * P, :], z_sb[:])

                        if i == int(gstart[g] + sizes[g]) - 1:
                            nc.gpsimd.collective_compute(
                                "AllGather", mybir.AluOpType.bypass,
                                replica_groups=[list(range(N_CORES))],
                                ins=[zs_d[l][g][:, :]],
                                outs=[zf_d[l][int(grp_off[g]):
                                              int(grp_off[g])
                                              + N_CORES * int(sizes[g]) * P,
                                              :]])

            # ---- decode: 64 buckets by (a-group, a-parity, b-group,
            # b-parity); each gather fetches a 2-row (256B) bf16 span
            z3 = zf_d[2]
            EW = 2 * out_c  # gathered width (two z3 rows)
            res = cst.tile([P, T], f32)
            tbase = 0
            for k in bucket_order():
                tk = int(tcnt[k])
                pb_ = k % 2
                gb_ = (k // 2) % N_GROUPS
                pa_ = (k // (2 * N_GROUPS)) % 2
                ga_ = k // (2 * N_GROUPS * 2)
                ta = z3[int(grp_off[ga_]):
                        int(grp_off[ga_]) + N_CORES * int(sizes[ga_]) * P, :]
                tb_ = z3[int(grp_off[gb_]):
                         int(grp_off[gb_]) + N_CORES * int(sizes[gb_]) * P, :]
                a_tab = ta.rearrange("(a two) f -> a (two f)", two=2)
                b_tab = tb_.rearrange("(a two) f -> a (two f)", two=2)
                for t0 in range(0, tk, DTC):
                    tc_ = min(DTC, tk - t0)
                    tb = tbase + t0
                    ga = dp.tile([P, DTC * EW], bf16, tag="ga")
                    gb = dp.tile([P, DTC * EW], bf16, tag="gb")
                    nc.gpsimd.dma_gather(
                        out_ap=ga[:, 0:tc_ * EW].rearrange(
                            "p (c f) -> p c f", c=tc_),
                        in_ap=a_tab,
                        idxs_ap=la_sb[:, tb * 8:(tb + tc_) * 8],
                        num_idxs=tc_ * P, num_idxs_reg=tc_ * P,
                        elem_size=EW,
                        single_packet=False, queue_num=next_q())
                    nc.gpsimd.dma_gather(
                        out_ap=gb[:, 0:tc_ * EW].rearrange(
                            "p (c f) -> p c f", c=tc_),
                        in_ap=b_tab,
                        idxs_ap=lb_sb[:, tb * 8:(tb + tc_) * 8],
                        num_idxs=tc_ * P, num_idxs_reg=tc_ * P,
                        elem_size=EW,
                        single_packet=False, queue_num=next_q())
                    g3a = ga[:, 0:tc_ * EW].rearrange("p (c f) -> p c f",
                                                      c=tc_)
                    g3b = gb[:, 0:tc_ * EW].rearrange("p (c f) -> p c f",
                                                      c=tc_)
                    prod = dp.tile([P, DTC * out_c], f32, tag="prod")
                    p3 = prod[:, 0:tc_ * out_c].rearrange(
                        "p (c f) -> p c f", c=tc_)
                    nc.vector.tensor_tensor(
                        out=p3,
                        in0=g3a[:, :, pa_ * out_c:(pa_ + 1) * out_c],
                        in1=g3b[:, :, pb_ * out_c:(pb_ + 1) * out_c],
                        op=mybir.AluOpType.mult)
                    nc.vector.tensor_reduce(
                        out=res[:, tb:tb + tc_],
                        in_=p3,
                        axis=mybir.AxisListType.X, op=mybir.AluOpType.add)
                tbase += tk
            nc.sync.dma_start(out_d[:, :], res[:])

    nc.finalize()
    return nc


# ---------------------------------------------------------------- entry point

def kernel(x, W1, b1, W2, b2, W3, b3, edge_index, edge_label_index):
    from concourse.bass_utils import run_bass_kernel_spmd

    x = np.ascontiguousarray(np.asarray(x, dtype=np.float32))
    n_nodes, in_c = x.shape
    hid_c = np.asarray(W2).shape[0]
    out_c = np.asarray(W3).shape[1]
    n_label = np.asarray(edge_label_index).shape[1]
    bpc = int(np.ceil(n_nodes / (N_CORES * P)))
    NPAD = N_CORES * bpc * P
    lay = make_layout(bpc)

    ed = prepare_edges(edge_index, n_nodes, bpc, lay)
    lb = prepare_labels(edge_label_index, n_label, bpc, lay)

    nc = build_bass(n_nodes, bpc, ed["cnt2d"], lb["tcnt"],
                    in_c, hid_c, out_c, lay, ed["kuse"])

    xb = np.zeros((NPAD, in_c), np.float32)
    xb[row_of(np.arange(n_nodes), bpc, lay)] = x
    xbq = xb.astype(BF16)

    common = {
        "W1": np.ascontiguousarray(np.asarray(W1, np.float32).astype(BF16)),
        "W2": np.ascontiguousarray(np.asarray(W2, np.float32).astype(BF16)),
        "W3": np.ascontiguousarray(np.asarray(W3, np.float32).astype(BF16)),
        "b1": np.ascontiguousarray(np.asarray(b1, np.float32).astype(BF16)),
        "b2": np.ascontiguousarray(np.asarray(b2, np.float32).astype(BF16)),
        "b3": np.ascontiguousarray(np.asarray(b3, np.float32).astype(BF16)),
    }
    in_maps = []
    C = ed["C"]
    for c in range(N_CORES):
        m = dict(common)
        xs = xbq[ed["grow"][c]].reshape(C, P, in_c).transpose(1, 0, 2)
        m["xs"] = np.ascontiguousarray(xs.reshape(P, C * in_c))
        m["eidx"] = np.ascontiguousarray(ed["eidx"][c])
        m["ind"] = np.ascontiguousarray(ed["inds"][c])
        m["la"] = np.ascontiguousarray(lb["la"][c])
        m["lb"] = np.ascontiguousarray(lb["lb"][c])
        in_maps.append(m)

    res = run_bass_kernel_spmd(nc, in_maps, core_ids=list(range(N_CORES)))

    out = np.zeros((n_label,), np.float32)
    for c in range(N_CORES):
        o = res.results[c]["out"]  # [P, T]
        flat = o.T.reshape(-1)
        lm = lb["labmap"][c]
        valid = lm >= 0
        out[lm[valid]] = flat[valid]
    return out
